# revision 43
# baseline (speedup 1.0000x reference)
"""Multi-head attention (B=2, S=2048, D=1024, H=16, causal mask) on 8 trn2
NeuronCores.  ~164us HW (vs 217-251us two-phase baseline).

Sharding: 2-way data parallel over batch x 4-way tensor parallel over head
groups (4 heads / core).  Core c handles batch c//4, head group c%4; each
core emits a y^T partial that the host sums (row-parallel TP reduction).

Single software-pipelined phase: projections, attention, and the output
projection are interleaved through a per-(chunk, head-pair) filler schedule
(`sched`) so the tensor engine works through Q/K/V projections of future
chunks and y-projections of past chunks while the scalar engine streams the
softmax exp (the second near-critical resource at ~88us vs PE ~133us).
Causality lets K/V projection of chunk c slide into chunk c itself (its
key-tiles are consumed late), which feeds the exp-heavy final chunks.

Layouts: everything on-chip is feature-major (no transposes anywhere); all
DRAM inputs are host-pre-swizzled so each DMA is one contiguous run per
partition (descriptor generation otherwise dominates: a [2,128]-transposed
constant load costs ~8us of trigger-queue time).  Softmax runs unnormalized
(unit-scale inputs cannot overflow), the denominator falls out of the A@V
matmul via a ones-column in V, the reciprocal rows use ln/exp on ScalarE
(same activation-table set as the softmax exp), and the per-partition
broadcast goes through one DRAM round trip -- except for the last head pair,
where a K=1 ones-matmul broadcasts through PSUM instead because the ~6us
DMA latency would sit on the critical tail.  Score matmuls of a head pair
land on different PE row-tiles (concurrent); exp covers both heads in one
instruction; causal-mask triangles are one stride-tricked multiply per tile.
Dummy matmuls bridge the startup DMA window and the tail normalize chain so
the HAM clock gate never drops to 4/8 mid-kernel.
"""

import os
import sys

import numpy as np

for _p in ("/opt/trn_rl_repo", "/root/.axon_site/_ro/trn_rl_repo"):
    if os.path.isdir(_p) and _p not in sys.path:
        sys.path.append(_p)

import ml_dtypes  # noqa: E402
from contextlib import ExitStack  # noqa: E402

import concourse.bass as bass  # noqa: E402
import concourse.tile as tile  # noqa: E402
from concourse import mybir  # noqa: E402

# ----- problem constants (hardcoded per contract) ---------------------------
B, S, D, H, DK = 2, 2048, 1024, 16, 64
NCORES = 8
TP = 4                      # head-parallel ways (per batch group)
EL = D // TP                # 256 local head dims = 4 heads
HL = H // TP                # 4 local heads
QC = 512                    # query-chunk (columns per attention pass)
NQC = S // QC               # 4
KT = 128                    # key tile (contraction tile for A@V)
NKT = S // KT               # 16
P = 128
NMT = D // P                # 8 output-feature tiles
SCALE = 1.0 / np.sqrt(DK)

F32 = mybir.dt.float32
F16 = mybir.dt.float16
F16NP = np.float16

# how many of the NMT y-projection evacuations per chunk run on ScalarE
# instead of DVE (load balancing between the two elementwise engines)
Y_ON_SCALAR = 0


# ----- host-side mask analysis ---------------------------------------------
class _KTile:
    __slots__ = ("kt", "s0", "s1", "muls", "first", "last")

    def __init__(self, kt, s0, s1, muls):
        self.kt, self.s0, self.s1, self.muls = kt, s0, s1, muls
        self.first = False
        self.last = False


def _mask_plan(mask2d):
    """mask2d: [S, S] ints, mask2d[q, k] (1 = attend).  Returns
    (plan, patterns) where plan[qc] is a list of _KTile and patterns is a
    fp16 array [n_pat, 128, 128] of transposed (k-major) mask blocks."""
    mT = (mask2d != 0).astype(np.float32).T          # [k, q]
    nqt = S // KT
    blk = mT.reshape(NKT, KT, nqt, KT).transpose(0, 2, 1, 3)
    sums = blk.sum(axis=(2, 3))
    patterns = []
    pat_idx = {}

    def pattern_id(kt, qt):
        key = blk[kt, qt].tobytes()
        if key not in pat_idx:
            pat_idx[key] = len(patterns)
            patterns.append(blk[kt, qt].astype(np.float16))
        return pat_idx[key]

    qt_per_qc = QC // KT
    plan = []
    for qc in range(NQC):
        tiles = []
        for kt in range(NKT):
            sub = sums[kt, qc * qt_per_qc:(qc + 1) * qt_per_qc]
            nz = [i for i in range(qt_per_qc) if sub[i] > 0]
            if not nz:
                continue
            s0, s1 = nz[0] * KT, (nz[-1] + 1) * KT
            tiles.append(_KTile(kt, s0, s1, None))
        if not tiles:
            raise ValueError(f"query chunk {qc} has no unmasked keys")
        u0 = min(t.s0 for t in tiles)
        u1 = max(t.s1 for t in tiles)
        tiles[0].s0, tiles[0].s1 = u0, u1
        tiles[0].first = True
        tiles[-1].last = True
        for t in tiles:
            muls = []
            for qt in range(t.s0 // KT, t.s1 // KT):
                full = sums[t.kt, qc * qt_per_qc + qt]
                if full != KT * KT:
                    muls.append((qt, pattern_id(t.kt, qc * qt_per_qc + qt)))
            t.muls = muls
        plan.append(tiles)
    pats = np.stack(patterns) if patterns else np.zeros((1, KT, KT), np.float16)
    return plan, pats


# ----- TileContext with a codegen-safe exit drain ---------------------------
# The stock kernel-tail drain carries one semaphore wait per engine/queue the
# kernel touched; CoreV3 codegen rejects instructions with more than two
# waits.  Split the waits across preceding sync-engine nops.
class _TileContext(tile.TileContext):
    def _drain_and_barrier(self, tick_clock, wait_clock):
        from concourse.vector_clock import ScopedClock
        nc = self.nc
        probe = nc.sync.nop()
        wait_clock.add_sem_waits(
            probe.ins, ScopedClock({None: tick_clock.global_clock}))
        si = probe.ins.sync_info
        waits = list(si.on_wait) if si and si.on_wait else []
        if len(waits) > 1:
            probe.ins.sync_info = mybir.SyncInfo(
                on_wait=waits[:1], on_update=list(si.on_update or []))
            for w in waits[1:]:
                n = nc.sync.nop()
                n.ins.sync_info = mybir.SyncInfo(on_wait=[w], on_update=[])
        nc.sync.drain()
        nc.all_engine_barrier()
        assert self.sems is not None
        popped = nc._tile_sem_poison_stack.pop()
        assert popped is self._sem_poison
        nc.clear_and_free_semaphores(list(self.sems.allocated().values()))
        nc.all_engine_barrier()


# The same wait-count limit applies to ordinary engine instructions under
# this walrus build: hoist all but one wait of every instruction onto
# preceding same-engine no-ops.
def _legalize_waits(nc, limit=1):
    for bb in nc.main_func.blocks:
        insts = list(bb.instructions)
        out = []
        for inst in insts:
            si = inst.sync_info
            waits = list(si.on_wait) if si and si.on_wait else []
            if len(waits) > limit:
                for w in waits[:-limit]:
                    nop = mybir.InstNoOp(
                        name=nc.get_next_instruction_name(), ins=[], outs=[])
                    nop.engine = inst.engine
                    nop.sync_info = mybir.SyncInfo(on_wait=[w], on_update=[])
                    nc.register_instruction(nop, overwrite=True)
                    out.append(nop)
                inst.sync_info = mybir.SyncInfo(
                    on_wait=waits[-limit:],
                    on_update=list(si.on_update or []))
            out.append(inst)
        bb.instructions = out


# ----- the bass program -----------------------------------------------------
def build_program(plan, n_pat):
    nc = bass.Bass(num_devices=NCORES)

    # all inputs are host-pre-swizzled so every DMA is one contiguous run
    # per partition (descriptor-generation time dominates small/strided
    # transfers on the trigger queues)
    xqT = nc.dram_tensor("xqT", [P, NQC, 8, QC], F16, kind="ExternalInput")
    xkT = nc.dram_tensor("xkT", [P, NQC, 8, QC], F16, kind="ExternalInput")
    xvT = nc.dram_tensor("xvT", [P, NQC, 8, QC], F16, kind="ExternalInput")
    wqT = nc.dram_tensor("wqT", [P, 8, EL], F16, kind="ExternalInput")
    wkT = nc.dram_tensor("wkT", [P, 8, EL], F16, kind="ExternalInput")
    wvT = nc.dram_tensor("wvT", [P, 8, EL], F16, kind="ExternalInput")
    woT = nc.dram_tensor("woT", [P, 2, D], F16, kind="ExternalInput")
    bq2 = nc.dram_tensor("bq2", [P, 2], F32, kind="ExternalInput")
    bk2 = nc.dram_tensor("bk2", [P, 2], F32, kind="ExternalInput")
    ybias = nc.dram_tensor("ybias", [P, NMT], F32, kind="ExternalInput")
    pats = nc.dram_tensor("pats", [P, n_pat, KT], F16, kind="ExternalInput")
    yT = nc.dram_tensor("yT", [D, S], F16, kind="ExternalOutput")

    with ExitStack() as ctx:
        tc = ctx.enter_context(_TileContext(nc))
        singles = ctx.enter_context(tc.tile_pool(name="singles", bufs=1))

        # --- persistent SBUF state ---
        wq_sb = singles.tile([P, 8, EL], F16)
        wk_sb = singles.tile([P, 8, EL], F16)
        wv_sb = singles.tile([P, 8, EL], F16)
        wo_sb = singles.tile([P, 2, D], F16)
        woB = singles.tile([DK, D], F16)
        bq_sb = singles.tile([P, 2], F32)
        bk_sb = singles.tile([P, 2], F32)
        yb_sb = singles.tile([P, NMT], F32)
        pat_sb = singles.tile([P, n_pat, KT], F16)
        Qt = singles.tile([P, 2, S], F16)     # [e-within-tile, e-tile, t]
        Kt = singles.tile([P, 2, S], F16)
        Vaug = singles.tile([P, NKT, HL, DK + 1], F16)

        xin = ctx.enter_context(tc.tile_pool(name="xin", bufs=12))
        x_ch = {}   # (name, chunk) -> sbuf tile

        # --- prologue DMAs, ordered by first use ---
        # a large DMA occupies its trigger queue for roughly the transfer
        # time, so the chunk-0 operands are split in halves across the
        # gpsimd and sync queues to halve their arrival latency
        xr = {"q": xqT, "k": xkT, "v": xvT}
        for nm in ("q", "k", "v"):
            x_ch[(nm, 0)] = xin.tile([P, 8, QC], F16, tag="xch", name=f"x_{nm}0")
        nc.sync.dma_start(out=wq_sb[:], in_=wqT[:, :, :])
        nc.gpsimd.dma_start(out=x_ch[("q", 0)][:, 0:4, :],
                            in_=xr["q"][:, 0, 0:4, :])
        nc.scalar.dma_start(out=x_ch[("k", 0)][:, 0:4, :],
                            in_=xr["k"][:, 0, 0:4, :])
        nc.sync.dma_start(out=x_ch[("q", 0)][:, 4:8, :],
                          in_=xr["q"][:, 0, 4:8, :])
        nc.sync.dma_start(out=wk_sb[:], in_=wkT[:, :, :])
        nc.gpsimd.dma_start(out=x_ch[("k", 0)][:, 4:8, :],
                            in_=xr["k"][:, 0, 4:8, :])
        nc.sync.dma_start(out=bq_sb[:], in_=bq2[:, :])
        nc.sync.dma_start(out=bk_sb[:], in_=bk2[:, :])
        nc.gpsimd.dma_start(out=x_ch[("v", 0)][:],
                            in_=xr["v"][:, 0, :, :])
        nc.sync.dma_start(out=wv_sb[:], in_=wvT[:, :, :])
        nc.sync.dma_start(out=pat_sb[:], in_=pats[:, :, :])
        for tci in range(1, NQC):
            for nm in ("q", "k", "v"):
                t = xin.tile([P, 8, QC], F16, tag="xch", name=f"x_{nm}{tci}")
                nc.sync.dma_start(out=t[:], in_=xr[nm][:, tci, :, :])
                x_ch[(nm, tci)] = t
            if tci == 1:
                nc.sync.dma_start(out=wo_sb[:], in_=woT[:, :, :])
                nc.sync.dma_start(out=yb_sb[:], in_=ybias[:, :])
                nc.gpsimd.dma_start(out=woB[:], in_=wo_sb[DK:P, 1, :])

        nc.vector.memset(Vaug[:, :, :, DK:DK + 1], 1.0)

        # activation-table warm: touch Exp AND Ln so the combined
        # natural_log_exp set loads once, during the prologue DMA window
        # (the tail normalize uses Ln on the otherwise-idle ScalarE)
        warm = singles.tile([P, 1], F32)
        nc.vector.memset(warm[:], 1.0)
        nc.scalar.activation(out=warm[0:1, :], in_=warm[0:1, :],
                             func=mybir.ActivationFunctionType.Exp)
        nc.scalar.activation(out=warm[0:1, :], in_=warm[0:1, :],
                             func=mybir.ActivationFunctionType.Ln)

        # --- pools ---
        pjps = ctx.enter_context(tc.tile_pool(name="pjps", bufs=2, space="PSUM"))
        spool = ctx.enter_context(tc.tile_pool(name="spool", bufs=2, space="PSUM"))
        avy = ctx.enter_context(tc.tile_pool(name="avy", bufs=1, space="PSUM"))
        ptp = ctx.enter_context(tc.tile_pool(name="ptp", bufs=6))
        xtp = ctx.enter_context(tc.tile_pool(name="xtp", bufs=2))
        nrm = ctx.enter_context(tc.tile_pool(name="nrm", bufs=2))
        ysb = ctx.enter_context(tc.tile_pool(name="ysb", bufs=4))
        prt = ctx.enter_context(tc.tile_pool(name="prt", bufs=8))
        dbp = ctx.enter_context(tc.tile_pool(name="dbp", bufs=2, space="DRAM"))

        # HAM warm-up: ~18 matmuls on a zeroed tile keep the PE busy while
        # the first real operands stream in, so the clock gate is already
        # 8/8 when projection work starts.
        wz = singles.tile([P, QC], F16)
        nc.vector.memset(wz[:], 0.0)
        ones1 = singles.tile([1, DK], F16)
        nc.vector.memset(ones1[:], 1.0)
        wp = pjps.tile([P, QC], F32, tag="pj", name="warmps")
        for r in range(15):
            nc.tensor.matmul(wp[:], lhsT=wz[:, 0:P], rhs=wz[:],
                             start=True, stop=True)

        yTr = yT.rearrange("(a p) t -> p a t", p=P)

        # --- filler generators (one call = one PE psum-group + its evac) ---
        def proj_group_q(tci, et, dst, b_sb, w_sb, nm):
            tsl = slice(tci * QC, (tci + 1) * QC)
            ps = pjps.tile([P, QC], F32, tag="pj", name=f"ps_{nm}{tci}{et}")
            xc = x_ch[(nm, tci)]
            for ft in range(8):
                nc.tensor.matmul(ps[:],
                                 lhsT=w_sb[:, ft, et * P:(et + 1) * P],
                                 rhs=xc[:, ft, :],
                                 start=(ft == 0), stop=(ft == 7))
            nc.vector.tensor_scalar_add(out=dst[:, et, tsl], in0=ps[:],
                                        scalar1=b_sb[:, et:et + 1])

        def proj_group_v(tci, pair):
            # two key-tiles (pair=0 -> tt 0,1; pair=1 -> tt 2,3) in one
            # psum bank, evacuated by a single strided cast
            ps = pjps.tile([P, 2, EL], F32, tag="pj", name=f"ps_v{tci}{pair}")
            xc = x_ch[("v", tci)]
            for tt in range(2):
                for ft in range(8):
                    nc.tensor.matmul(
                        ps[:, tt, :],
                        lhsT=xc[:, ft, (2 * pair + tt) * P:(2 * pair + tt + 1) * P],
                        rhs=wv_sb[:, ft, :],
                        start=(ft == 0), stop=(ft == 7))
            ktg = tci * 4 + 2 * pair
            src = ps[:].rearrange("p a (h e) -> p a h e", h=HL)
            nc.vector.tensor_copy(out=Vaug[:, ktg:ktg + 2, :, 0:DK], in_=src)

        def f_q(tci, et):
            return lambda: proj_group_q(tci, et, Qt, bq_sb, wq_sb, "q")

        def f_k(tci, et):
            return lambda: proj_group_q(tci, et, Kt, bk_sb, wk_sb, "k")

        def f_v(tci, pair):
            return lambda: proj_group_v(tci, pair)

        def yproj_group(qc, mt, xTt):
            yp = pjps.tile([P, QC], F32, tag="pj", name=f"yp{qc}{mt}")
            for ct in range(2):
                nc.tensor.matmul(yp[:],
                                 lhsT=wo_sb[:, ct, mt * P:(mt + 1) * P],
                                 rhs=xTt[:, ct, :],
                                 start=(ct == 0), stop=(ct == 1))
            ys = ysb.tile([P, QC], F16, tag="ys", name=f"ys{qc}{mt}")
            if mt < Y_ON_SCALAR:
                nc.scalar.activation(out=ys[:], in_=yp[:],
                                     func=mybir.ActivationFunctionType.Identity,
                                     bias=yb_sb[:, mt:mt + 1])
            else:
                nc.vector.tensor_scalar_add(out=ys[:], in0=yp[:],
                                            scalar1=yb_sb[:, mt:mt + 1])
            nc.sync.dma_start(out=yTr[:, mt, qc * QC:(qc + 1) * QC], in_=ys[:])

        def f_y(qc, mt):
            return lambda: yproj_group(qc, mt, xTts[qc])

        # --- attention for one (qc, hp) with PE filler interleave ---
        def attention_hp(qc, hp, xTt, fillers, pre=None):
            tiles = plan[qc]
            et = hp
            ntile = len(tiles)
            av = avy.tile([P, 2, QC], F32, tag="avy", name=f"av{qc}{hp}")
            ptbs = []

            def emit_av(ti, t):
                for hh in range(2):
                    nc.tensor.matmul(
                        av[0:DK + 1, hh, t.s0:t.s1],
                        lhsT=Vaug[:, t.kt, 2 * hp + hh, :],
                        rhs=ptbs[ti][:, hh, t.s0:t.s1],
                        start=t.first, stop=t.last,
                        skip_group_check=True)

            emitted = 0
            for ti, t in enumerate(tiles):
                ps = spool.tile([P, 2, QC], F32, tag="s",
                                name=f"s{qc}{hp}{ti}")
                for hh in range(2):
                    po = hh * DK
                    nc.tensor.matmul(
                        ps[:, hh, t.s0:t.s1],
                        lhsT=Kt[po:po + DK, et, t.kt * KT:(t.kt + 1) * KT],
                        rhs=Qt[po:po + DK, et, qc * QC + t.s0:qc * QC + t.s1],
                        start=True, stop=True)
                if ti > 0:
                    emit_av(ti - 1, tiles[ti - 1])
                # one exp covering both heads' identical column ranges
                pt = ptp.tile([P, 2, QC], F16, tag="pt", name=f"pt{qc}{hp}{ti}")
                ptbs.append(pt)
                nc.scalar.activation(
                    out=pt[:, :, t.s0:t.s1], in_=ps[:, :, t.s0:t.s1],
                    func=mybir.ActivationFunctionType.Exp, scale=float(SCALE))
                if ti == 1 and pre is not None:
                    pre()
                    pre = None
                # mixed blocks: one masked multiply for both heads
                for qt, pid in t.muls:
                    sl = slice(qt * KT, (qt + 1) * KT)
                    pm = pat_sb[:, pid, :].rearrange(
                        "p k -> p () k").broadcast_to((P, 2, KT))
                    nc.vector.tensor_tensor(
                        out=pt[:, :, sl], in0=pt[:, :, sl], in1=pm,
                        op=mybir.AluOpType.mult)
                # PE filler budget: spread fillers evenly across tiles
                want = len(fillers) * (ti + 1) // ntile
                while emitted < want:
                    fillers[emitted]()
                    emitted += 1
            emit_av(ntile - 1, tiles[-1])
            return av

        # --- normalize: copy av out now; reciprocal+broadcast deferred ---
        bcs = {}

        def norm_copy(qc, hp, av):
            """copy av out of PSUM (frees the accumulator banks for the next
            head pair); the denominator row rides along in the copy."""
            cpb = nrm.tile([DK + 1, 2, QC], F32, tag="cp",
                           name=f"cp{qc}{hp}")
            nc.vector.tensor_copy(out=cpb[:], in_=av[0:DK + 1, :, :])
            return cpb

        def norm_rr(qc, hp, cpb):
            """reciprocal rows via ScalarE ln/exp (same activation-table set
            as the softmax exp), broadcast across partitions via one DRAM
            round-trip.  Emitted two tiles into the NEXT head-pair so it
            does not interrupt the exp stream at the boundary."""
            rr = nrm.tile([1, 2, QC], F32, tag="rr", name=f"rr{qc}{hp}")
            nc.scalar.activation(out=rr[:], in_=cpb[DK:DK + 1, :, :],
                                 func=mybir.ActivationFunctionType.Ln)
            nc.scalar.activation(out=rr[:], in_=rr[:],
                                 func=mybir.ActivationFunctionType.Exp,
                                 scale=-1.0)
            dnb = dbp.tile([P, 8], F32, tag="dnb", name=f"dnb{qc}{hp}")
            nc.gpsimd.dma_start(
                out=dnb[:].rearrange("p j -> () (p j)"),
                in_=rr[:].rearrange("o h q -> o (h q)"))
            bc = nrm.tile([DK, 2, QC], F32, tag="bc", name=f"bc{qc}{hp}")
            dnf = dnb[:].rearrange("p j -> () (p j)").rearrange(
                "o (h q) -> o h q", h=2)
            nc.gpsimd.dma_start(out=bc[:], in_=dnf[0:1].partition_broadcast(DK))
            bcs[(qc, hp)] = bc

        def norm_mult(qc, hp, xTt, in0s, bc):
            et = hp
            nc.vector.tensor_tensor(
                out=xTt[0:DK, et, :], in0=in0s[0], in1=bc[:, 0, :],
                op=mybir.AluOpType.mult)
            tmp = nrm.tile([DK, QC], F16, tag="tmp", name=f"tm{qc}{hp}")
            nc.vector.tensor_tensor(
                out=tmp[:], in0=in0s[1], in1=bc[:, 1, :],
                op=mybir.AluOpType.mult)
            nc.scalar.dma_start(out=xTt[DK:P, et, :], in_=tmp[:])

        # --- global filler schedule -----------------------------------
        # Q proj of chunk c must land before qc c starts; K proj of chunk c
        # before its scores reach key-tile 4c; V proj of chunk c before its
        # A@V reaches key-tile 4c.  That lets K/V work slide late, feeding
        # the exp-bound final chunks where attention alone can't keep the
        # PE busy.  yproj(c) slides anywhere after chunk c's normalize.
        xTts = {}
        sched = {
            (0, 0): [f_v(0, 0), f_v(0, 1), f_q(0, 1), f_k(0, 1),
                     f_q(1, 0), f_q(1, 1)],
            (0, 1): [f_k(1, 0), f_k(1, 1)],
            (1, 0): [f_v(1, 0), f_v(1, 1), f_q(2, 0), f_q(2, 1)],
            (1, 1): [f_k(2, 0), f_k(2, 1), f_y(0, 0), f_y(0, 1),
                     f_y(0, 2), f_y(0, 3), f_y(0, 4), f_y(0, 5)],
            (2, 0): [f_v(2, 0), f_v(2, 1), f_q(3, 0), f_q(3, 1),
                     f_y(0, 6), f_y(0, 7)],
            (2, 1): [f_k(3, 0), f_k(3, 1), f_y(1, 0), f_y(1, 1),
                     f_y(1, 2), f_y(1, 3)],
            (3, 0): [f_v(3, 0), f_v(3, 1), f_y(1, 4), f_y(1, 5),
                     f_y(1, 6), f_y(1, 7)],
        }

        # qc3.hp1 additionally computes the ct=0 halves of yproj(3) into
        # SBUF partials (the tail then only needs the ct=1 matmuls), and
        # the (3,0) normalize multiply rides along as a filler once its
        # broadcast has landed
        nm30 = {}
        parts3 = {}

        def f_mult30():
            def go():
                xTt, in0s, _ = nm30["args"]
                norm_mult(3, 0, xTt, in0s, bcs[(3, 0)])
            return go

        def f_ypart3(mt):
            def go():
                yp = pjps.tile([P, QC], F32, tag="pj", name=f"yp3a{mt}")
                nc.tensor.matmul(yp[:],
                                 lhsT=wo_sb[:, 0, mt * P:(mt + 1) * P],
                                 rhs=xTts[3][:, 0, :],
                                 start=True, stop=True)
                part = prt.tile([P, QC], F16, tag="part", name=f"part{mt}")
                nc.vector.tensor_copy(out=part[:], in_=yp[:])
                parts3[mt] = part
            return go

        sched[(3, 1)] = ([f_y(2, mt) for mt in range(6)] + [f_mult30()] +
                         [f_y(2, 6), f_y(2, 7)] +
                         [f_ypart3(mt) for mt in range(NMT)])

        # minimal prologue projections: just what qc0.hp0 needs up front
        proj_group_q(0, 0, Qt, bq_sb, wq_sb, "q")
        proj_group_q(0, 0, Kt, bk_sb, wk_sb, "k")

        # --- main pipelined loop ---
        # a normalize multiply waits on its 4-hop DMA chain, so it is
        # emitted one half-chunk after its norm_evac: (qc-1,hp1)'s multiply
        # goes between (qc,hp0) and (qc,hp1); (qc,hp0)'s goes after
        # (qc,hp1).  yproj fillers for a chunk only appear after its hp1
        # multiply has been emitted.
        prev_hp1 = None          # (qc, 1, xTt, cpb) of the previous chunk
        pend_rr = None           # deferred ln/exp+broadcast thunk
        for qc in range(NQC):
            xTt = xtp.tile([P, 2, QC], F16, tag="xT", name=f"xT{qc}")
            xTts[qc] = xTt

            av0 = attention_hp(qc, 0, xTt, sched[(qc, 0)], pre=pend_rr)
            cpb0 = norm_copy(qc, 0, av0)
            pend_rr = (lambda q=qc, c=cpb0: norm_rr(q, 0, c))
            if prev_hp1 is not None:
                pq, ph, pxt, pcpb = prev_hp1
                norm_mult(pq, ph, pxt,
                          [pcpb[0:DK, 0, :], pcpb[0:DK, 1, :]], bcs[(pq, ph)])
                prev_hp1 = None
            if qc == NQC - 1:
                nm30["args"] = (xTt, [cpb0[0:DK, 0, :], cpb0[0:DK, 1, :]],
                                None)

            av1 = attention_hp(qc, 1, xTt, sched[(qc, 1)], pre=pend_rr)
            pend_rr = None
            if qc < NQC - 1:
                cpb1 = norm_copy(qc, 1, av1)
                pend_rr = (lambda q=qc, c=cpb1: norm_rr(q, 1, c))
                norm_mult(qc, 0, xTt,
                          [cpb0[0:DK, 0, :], cpb0[0:DK, 1, :]], bcs[(qc, 0)])
                prev_hp1 = (qc, 1, xTt, cpb1)
            else:
                # tail normalize: reciprocal rows straight to fp16, then
                # broadcast with a K=1 ones-matmul into PSUM instead of the
                # ~6us DRAM round trip (the PE is idle here anyway)
                cpb = nrm.tile([DK + 1, 2, QC], F32, tag="cp", name="cpT")
                nc.vector.tensor_copy(out=cpb[:], in_=av1[0:DK + 1, :, :])
                rr = nrm.tile([1, 2, QC], F32, tag="rr", name="rrT")
                nc.scalar.activation(out=rr[:], in_=av1[DK:DK + 1, :, :],
                                     func=mybir.ActivationFunctionType.Ln)
                rr16 = nrm.tile([1, 2, QC], F16, tag="rr16", name="rrT16")
                nc.scalar.activation(out=rr16[:], in_=rr[:],
                                     func=mybir.ActivationFunctionType.Exp,
                                     scale=-1.0)
                # keep the PE clock warm while ln/exp run
                wpt = pjps.tile([P, QC], F32, tag="pj", name="tailwarm")
                for r in range(8):
                    nc.tensor.matmul(wpt[:], lhsT=wz[:, 0:P], rhs=wz[:],
                                     start=True, stop=True)
                bcp = avy.tile([P, 2, QC], F32, tag="avy", name="bcT")
                for hh in range(2):
                    nc.tensor.matmul(bcp[0:DK, hh, :], lhsT=ones1[:],
                                     rhs=rr16[:, hh, :], start=True, stop=True)
                for r in range(8):
                    nc.tensor.matmul(wpt[:], lhsT=wz[:, 0:P], rhs=wz[:],
                                     start=True, stop=True)
                tail_tmp = nrm.tile([DK, QC], F16, tag="tmp", name="tmT")
                nc.vector.tensor_tensor(
                    out=tail_tmp[:], in0=cpb[0:DK, 1, :], in1=bcp[0:DK, 1, :],
                    op=mybir.AluOpType.mult)
                nc.vector.tensor_tensor(
                    out=xTt[0:DK, 1, :], in0=cpb[0:DK, 0, :],
                    in1=bcp[0:DK, 0, :], op=mybir.AluOpType.mult)

        # tail: ct=1 matmuls of the last chunk's y projection, folded with
        # the precomputed ct=0 partials
        xT3 = xTts[NQC - 1]
        ysp = None
        for mt in range(NMT):
            if mt % 2 == 0:
                yp = pjps.tile([P, QC], F32, tag="pj", name=f"yp3b{mt}")
                ysp = ysb.tile([P, 2, QC], F16, tag="ys2", name=f"ys3p{mt}")
            else:
                ypw = spool.tile([P, 2, QC], F32, tag="s", name=f"yp3s{mt}")
                yp = ypw[:, 0, :]
            nc.tensor.matmul(yp[:],
                             lhsT=wo_sb[0:DK, 1, mt * P:(mt + 1) * P],
                             rhs=xT3[0:DK, 1, :],
                             start=True, stop=False)
            nc.tensor.matmul(yp[:],
                             lhsT=woB[:, mt * P:(mt + 1) * P],
                             rhs=tail_tmp[:],
                             start=False, stop=True)
            nc.vector.scalar_tensor_tensor(
                out=ysp[:, mt % 2, :], in0=yp[:], scalar=yb_sb[:, mt:mt + 1],
                in1=parts3[mt][:], op0=mybir.AluOpType.add,
                op1=mybir.AluOpType.add)
            if mt % 2 == 1:
                nc.sync.dma_start(
                    out=yTr[:, mt - 1:mt + 1, (NQC - 1) * QC:NQC * QC],
                    in_=ysp[:])

    _legalize_waits(nc)
    return nc


# ----- SPMD runner ----------------------------------------------------------
_NEFF_MEMO = {}


def _install_memo_hook():
    import libneuronxla
    from concourse.bass2jax import install_neuronx_cc_hook

    install_neuronx_cc_hook()
    inner = libneuronxla.neuronx_cc
    if getattr(inner, "_is_memo_hook", False):
        return

    def memo_hook(code, code_format, platform_version, file_prefix):
        import hashlib
        key = hashlib.sha256(bytes(code)).hexdigest()
        if key not in _NEFF_MEMO:
            _NEFF_MEMO[key] = inner(code, code_format, platform_version,
                                    file_prefix)
        return _NEFF_MEMO[key]

    memo_hook._is_memo_hook = True
    libneuronxla.neuronx_cc = memo_hook


def run_spmd(nc, in_maps):
    import jax
    from concourse.bass2jax import _bass_exec_p

    _install_memo_hook()
    n_cores = len(in_maps)
    partition_name = (nc.partition_id_tensor.name
                      if nc.partition_id_tensor is not None else None)
    in_names, out_names, out_avals = [], [], []
    for alloc in nc.m.functions[0].allocations:
        if not isinstance(alloc, mybir.MemoryLocationSet):
            continue
        name = alloc.memorylocations[0].name
        if alloc.kind == "ExternalInput":
            if name != partition_name:
                in_names.append(name)
        elif alloc.kind == "ExternalOutput":
            out_names.append(name)
            out_avals.append(jax.core.ShapedArray(
                tuple(alloc.tensor_shape), mybir.dt.np(alloc.dtype)))
    bind_in_names = tuple(in_names +
                          ([partition_name] if partition_name else []))

    def _body(*args):
        return tuple(_bass_exec_p.bind(
            *args, out_avals=tuple(out_avals), in_names=bind_in_names,
            out_names=tuple(out_names), lowering_input_output_aliases=(),
            sim_require_finite=True, sim_require_nnan=True, nc=nc))

    devices = jax.devices()[:n_cores]
    f = jax.jit(_body)
    futs = []
    for c in range(n_cores):
        args = [jax.device_put(np.asarray(in_maps[c][nm]), devices[c])
                for nm in in_names]
        if partition_name:
            args.append(jax.device_put(np.array([[c]], np.uint32), devices[c]))
        futs.append(f(*args))
    return [{nm: np.asarray(futs[c][i]) for i, nm in enumerate(out_names)}
            for c in range(n_cores)]


# ----- host wrapper ---------------------------------------------------------
_CACHE = {}


def _get_program(mask):
    key = mask.tobytes()
    if key not in _CACHE:
        plan, pats = _mask_plan(mask)
        nc = build_program(plan, pats.shape[0])
        _CACHE[key] = (nc, pats)
    return _CACHE[key]


def _sw_x(xT):
    """[D, S] -> [P, NQC, 8, QC] so each (partition, chunk) is contiguous"""
    return np.ascontiguousarray(
        xT.reshape(8, P, NQC, QC).transpose(1, 2, 0, 3))


def _sw_w(wT, a):
    """[a*P, cols] -> [P, a, cols]"""
    return np.ascontiguousarray(
        wT.reshape(a, P, wT.shape[1]).transpose(1, 0, 2))


def make_in_maps(q, k, v, mask, wq, bq, wk, bk, wv, bv, wo, bo, pats):
    q, k, v = (np.asarray(a, np.float32) for a in (q, k, v))
    in_maps = []
    for c in range(NCORES):
        b, g = divmod(c, TP)
        sl = slice(g * EL, (g + 1) * EL)
        woT_g = np.ascontiguousarray(wo[:, sl].T)        # [EL, D]
        in_maps.append({
            "xqT": _sw_x(q[b].T.astype(F16NP)),
            "xkT": _sw_x(k[b].T.astype(F16NP)),
            "xvT": _sw_x(v[b].T.astype(F16NP)),
            "wqT": _sw_w(wq[sl, :].T.astype(F16NP), 8),
            "wkT": _sw_w(wk[sl, :].T.astype(F16NP), 8),
            "wvT": _sw_w(wv[sl, :].T.astype(F16NP), 8),
            "woT": _sw_w(woT_g.astype(F16NP), 2),
            "bq2": np.ascontiguousarray(bq[sl].reshape(2, P).T),
            "bk2": np.ascontiguousarray(bk[sl].reshape(2, P).T),
            "ybias": np.ascontiguousarray(
                (bv[sl].astype(np.float64) @ woT_g.astype(np.float64))
                .astype(np.float32).reshape(NMT, P).T),
            "pats": np.ascontiguousarray(pats.transpose(1, 0, 2)),
        })
    return in_maps


def assemble_output(results, bo):
    y = np.empty((B, S, D), np.float32)
    for b in range(B):
        acc = results[b * TP]["yT"].astype(np.float32)
        for g in range(1, TP):
            acc = acc + results[b * TP + g]["yT"].astype(np.float32)
        y[b] = acc.T + np.asarray(bo, np.float32)[None, :]
    return y


def kernel(q, k, v, mask, wq, bq, wk, bk, wv, bv, wo, bo):
    mask2d = np.asarray(mask).reshape(S, S)
    nc, pats = _get_program(mask2d)
    in_maps = make_in_maps(q, k, v, mask2d, wq, bq, wk, bk, wv, bv, wo, bo, pats)
    return assemble_output(run_spmd(nc, in_maps), bo)


# revision 44
# speedup vs baseline: 1.0053x; 1.0053x over previous
"""Multi-head attention (B=2, S=2048, D=1024, H=16, causal mask) on 8 trn2
NeuronCores.  ~164us HW (vs 217-251us two-phase baseline).

Sharding: 2-way data parallel over batch x 4-way tensor parallel over head
groups (4 heads / core).  Core c handles batch c//4, head group c%4; each
core emits a y^T partial that the host sums (row-parallel TP reduction).

Single software-pipelined phase: projections, attention, and the output
projection are interleaved through a per-(chunk, head-pair) filler schedule
(`sched`) so the tensor engine works through Q/K/V projections of future
chunks and y-projections of past chunks while the scalar engine streams the
softmax exp (the second near-critical resource at ~88us vs PE ~133us).
Causality lets K/V projection of chunk c slide into chunk c itself (its
key-tiles are consumed late), which feeds the exp-heavy final chunks.

Layouts: everything on-chip is feature-major (no transposes anywhere); all
DRAM inputs are host-pre-swizzled so each DMA is one contiguous run per
partition (descriptor generation otherwise dominates: a [2,128]-transposed
constant load costs ~8us of trigger-queue time).  Softmax runs unnormalized
(unit-scale inputs cannot overflow), the denominator falls out of the A@V
matmul via a ones-column in V, the reciprocal rows use ln/exp on ScalarE
(same activation-table set as the softmax exp), and the per-partition
broadcast goes through one DRAM round trip -- except for the last head pair,
where a K=1 ones-matmul broadcasts through PSUM instead because the ~6us
DMA latency would sit on the critical tail.  Score matmuls of a head pair
land on different PE row-tiles (concurrent); exp covers both heads in one
instruction; causal-mask triangles are one stride-tricked multiply per tile.
Dummy matmuls bridge the startup DMA window and the tail normalize chain so
the HAM clock gate never drops to 4/8 mid-kernel.
"""

import os
import sys

import numpy as np

for _p in ("/opt/trn_rl_repo", "/root/.axon_site/_ro/trn_rl_repo"):
    if os.path.isdir(_p) and _p not in sys.path:
        sys.path.append(_p)

import ml_dtypes  # noqa: E402
from contextlib import ExitStack  # noqa: E402

import concourse.bass as bass  # noqa: E402
import concourse.tile as tile  # noqa: E402
from concourse import mybir  # noqa: E402

# ----- problem constants (hardcoded per contract) ---------------------------
B, S, D, H, DK = 2, 2048, 1024, 16, 64
NCORES = 8
TP = 4                      # head-parallel ways (per batch group)
EL = D // TP                # 256 local head dims = 4 heads
HL = H // TP                # 4 local heads
QC = 512                    # query-chunk (columns per attention pass)
NQC = S // QC               # 4
KT = 128                    # key tile (contraction tile for A@V)
NKT = S // KT               # 16
P = 128
NMT = D // P                # 8 output-feature tiles
SCALE = 1.0 / np.sqrt(DK)

F32 = mybir.dt.float32
F16 = mybir.dt.float16
F16NP = np.float16

# how many of the NMT y-projection evacuations per chunk run on ScalarE
# instead of DVE (load balancing between the two elementwise engines)
Y_ON_SCALAR = 0


# ----- host-side mask analysis ---------------------------------------------
class _KTile:
    __slots__ = ("kt", "s0", "s1", "muls", "first", "last")

    def __init__(self, kt, s0, s1, muls):
        self.kt, self.s0, self.s1, self.muls = kt, s0, s1, muls
        self.first = False
        self.last = False


def _mask_plan(mask2d):
    """mask2d: [S, S] ints, mask2d[q, k] (1 = attend).  Returns
    (plan, patterns) where plan[qc] is a list of _KTile and patterns is a
    fp16 array [n_pat, 128, 128] of transposed (k-major) mask blocks."""
    mT = (mask2d != 0).astype(np.float32).T          # [k, q]
    nqt = S // KT
    blk = mT.reshape(NKT, KT, nqt, KT).transpose(0, 2, 1, 3)
    sums = blk.sum(axis=(2, 3))
    patterns = []
    pat_idx = {}

    def pattern_id(kt, qt):
        key = blk[kt, qt].tobytes()
        if key not in pat_idx:
            pat_idx[key] = len(patterns)
            patterns.append(blk[kt, qt].astype(np.float16))
        return pat_idx[key]

    qt_per_qc = QC // KT
    plan = []
    for qc in range(NQC):
        tiles = []
        for kt in range(NKT):
            sub = sums[kt, qc * qt_per_qc:(qc + 1) * qt_per_qc]
            nz = [i for i in range(qt_per_qc) if sub[i] > 0]
            if not nz:
                continue
            s0, s1 = nz[0] * KT, (nz[-1] + 1) * KT
            tiles.append(_KTile(kt, s0, s1, None))
        if not tiles:
            raise ValueError(f"query chunk {qc} has no unmasked keys")
        u0 = min(t.s0 for t in tiles)
        u1 = max(t.s1 for t in tiles)
        tiles[0].s0, tiles[0].s1 = u0, u1
        tiles[0].first = True
        tiles[-1].last = True
        for t in tiles:
            muls = []
            for qt in range(t.s0 // KT, t.s1 // KT):
                full = sums[t.kt, qc * qt_per_qc + qt]
                if full != KT * KT:
                    muls.append((qt, pattern_id(t.kt, qc * qt_per_qc + qt)))
            t.muls = muls
        plan.append(tiles)
    pats = np.stack(patterns) if patterns else np.zeros((1, KT, KT), np.float16)
    return plan, pats


# ----- TileContext with a codegen-safe exit drain ---------------------------
# The stock kernel-tail drain carries one semaphore wait per engine/queue the
# kernel touched; CoreV3 codegen rejects instructions with more than two
# waits.  Split the waits across preceding sync-engine nops.
class _TileContext(tile.TileContext):
    def _drain_and_barrier(self, tick_clock, wait_clock):
        from concourse.vector_clock import ScopedClock
        nc = self.nc
        probe = nc.sync.nop()
        wait_clock.add_sem_waits(
            probe.ins, ScopedClock({None: tick_clock.global_clock}))
        si = probe.ins.sync_info
        waits = list(si.on_wait) if si and si.on_wait else []
        if len(waits) > 1:
            probe.ins.sync_info = mybir.SyncInfo(
                on_wait=waits[:1], on_update=list(si.on_update or []))
            for w in waits[1:]:
                n = nc.sync.nop()
                n.ins.sync_info = mybir.SyncInfo(on_wait=[w], on_update=[])
        nc.sync.drain()
        nc.all_engine_barrier()
        assert self.sems is not None
        popped = nc._tile_sem_poison_stack.pop()
        assert popped is self._sem_poison
        nc.clear_and_free_semaphores(list(self.sems.allocated().values()))
        nc.all_engine_barrier()


# The same wait-count limit applies to ordinary engine instructions under
# this walrus build: hoist all but one wait of every instruction onto
# preceding same-engine no-ops.
def _legalize_waits(nc, limit=1):
    for bb in nc.main_func.blocks:
        insts = list(bb.instructions)
        out = []
        for inst in insts:
            si = inst.sync_info
            waits = list(si.on_wait) if si and si.on_wait else []
            if len(waits) > limit:
                for w in waits[:-limit]:
                    nop = mybir.InstNoOp(
                        name=nc.get_next_instruction_name(), ins=[], outs=[])
                    nop.engine = inst.engine
                    nop.sync_info = mybir.SyncInfo(on_wait=[w], on_update=[])
                    nc.register_instruction(nop, overwrite=True)
                    out.append(nop)
                inst.sync_info = mybir.SyncInfo(
                    on_wait=waits[-limit:],
                    on_update=list(si.on_update or []))
            out.append(inst)
        bb.instructions = out


# ----- the bass program -----------------------------------------------------
def build_program(plan, n_pat):
    nc = bass.Bass(num_devices=NCORES)

    # all inputs are host-pre-swizzled so every DMA is one contiguous run
    # per partition (descriptor-generation time dominates small/strided
    # transfers on the trigger queues)
    xqT = nc.dram_tensor("xqT", [P, NQC, 8, QC], F16, kind="ExternalInput")
    xkT = nc.dram_tensor("xkT", [P, NQC, 8, QC], F16, kind="ExternalInput")
    xvT = nc.dram_tensor("xvT", [P, NQC, 8, QC], F16, kind="ExternalInput")
    wqT = nc.dram_tensor("wqT", [P, 8, EL], F16, kind="ExternalInput")
    wkT = nc.dram_tensor("wkT", [P, 8, EL], F16, kind="ExternalInput")
    wvT = nc.dram_tensor("wvT", [P, 8, EL], F16, kind="ExternalInput")
    woT = nc.dram_tensor("woT", [P, 2, D], F16, kind="ExternalInput")
    bq2 = nc.dram_tensor("bq2", [P, 2], F32, kind="ExternalInput")
    bk2 = nc.dram_tensor("bk2", [P, 2], F32, kind="ExternalInput")
    ybias = nc.dram_tensor("ybias", [P, NMT], F32, kind="ExternalInput")
    pats = nc.dram_tensor("pats", [P, n_pat, KT], F16, kind="ExternalInput")
    yT = nc.dram_tensor("yT", [D, S], F16, kind="ExternalOutput")

    with ExitStack() as ctx:
        tc = ctx.enter_context(_TileContext(nc))
        singles = ctx.enter_context(tc.tile_pool(name="singles", bufs=1))

        # --- persistent SBUF state ---
        wq_sb = singles.tile([P, 8, EL], F16)
        wk_sb = singles.tile([P, 8, EL], F16)
        wv_sb = singles.tile([P, 8, EL], F16)
        wo_sb = singles.tile([P, 2, D], F16)
        woB = singles.tile([DK, D], F16)
        bq_sb = singles.tile([P, 2], F32)
        bk_sb = singles.tile([P, 2], F32)
        yb_sb = singles.tile([P, NMT], F32)
        pat_sb = singles.tile([P, n_pat, KT], F16)
        Qt = singles.tile([P, 2, S], F16)     # [e-within-tile, e-tile, t]
        Kt = singles.tile([P, 2, S], F16)
        Vaug = singles.tile([P, NKT, HL, DK + 1], F16)

        xin = ctx.enter_context(tc.tile_pool(name="xin", bufs=12))
        x_ch = {}   # (name, chunk) -> sbuf tile

        # --- prologue DMAs, ordered by first use ---
        # a large DMA occupies its trigger queue for roughly the transfer
        # time, so the chunk-0 operands are split in halves across the
        # gpsimd and sync queues to halve their arrival latency
        xr = {"q": xqT, "k": xkT, "v": xvT}
        for nm in ("q", "k", "v"):
            x_ch[(nm, 0)] = xin.tile([P, 8, QC], F16, tag="xch", name=f"x_{nm}0")
        nc.sync.dma_start(out=wq_sb[:], in_=wqT[:, :, :])
        nc.gpsimd.dma_start(out=x_ch[("q", 0)][:, 0:4, :],
                            in_=xr["q"][:, 0, 0:4, :])
        nc.scalar.dma_start(out=x_ch[("k", 0)][:, 0:4, :],
                            in_=xr["k"][:, 0, 0:4, :])
        nc.sync.dma_start(out=x_ch[("q", 0)][:, 4:8, :],
                          in_=xr["q"][:, 0, 4:8, :])
        nc.sync.dma_start(out=wk_sb[:], in_=wkT[:, :, :])
        nc.gpsimd.dma_start(out=x_ch[("k", 0)][:, 4:8, :],
                            in_=xr["k"][:, 0, 4:8, :])
        nc.sync.dma_start(out=bq_sb[:], in_=bq2[:, :])
        nc.sync.dma_start(out=bk_sb[:], in_=bk2[:, :])
        nc.gpsimd.dma_start(out=x_ch[("v", 0)][:],
                            in_=xr["v"][:, 0, :, :])
        nc.sync.dma_start(out=wv_sb[:], in_=wvT[:, :, :])
        nc.sync.dma_start(out=pat_sb[:], in_=pats[:, :, :])
        for tci in range(1, NQC):
            for nm in ("q", "k", "v"):
                t = xin.tile([P, 8, QC], F16, tag="xch", name=f"x_{nm}{tci}")
                nc.sync.dma_start(out=t[:], in_=xr[nm][:, tci, :, :])
                x_ch[(nm, tci)] = t
            if tci == 1:
                nc.sync.dma_start(out=wo_sb[:], in_=woT[:, :, :])
                nc.sync.dma_start(out=yb_sb[:], in_=ybias[:, :])
                nc.gpsimd.dma_start(out=woB[:], in_=wo_sb[DK:P, 1, :])

        nc.vector.memset(Vaug[:, :, :, DK:DK + 1], 1.0)

        # activation-table warm: touch Exp AND Ln so the combined
        # natural_log_exp set loads once, during the prologue DMA window
        # (the tail normalize uses Ln on the otherwise-idle ScalarE)
        warm = singles.tile([P, 1], F32)
        nc.vector.memset(warm[:], 1.0)
        nc.scalar.activation(out=warm[0:1, :], in_=warm[0:1, :],
                             func=mybir.ActivationFunctionType.Exp)
        nc.scalar.activation(out=warm[0:1, :], in_=warm[0:1, :],
                             func=mybir.ActivationFunctionType.Ln)

        # --- pools ---
        pjps = ctx.enter_context(tc.tile_pool(name="pjps", bufs=2, space="PSUM"))
        spool = ctx.enter_context(tc.tile_pool(name="spool", bufs=2, space="PSUM"))
        avy = ctx.enter_context(tc.tile_pool(name="avy", bufs=1, space="PSUM"))
        ptp = ctx.enter_context(tc.tile_pool(name="ptp", bufs=6))
        xtp = ctx.enter_context(tc.tile_pool(name="xtp", bufs=2))
        nrm = ctx.enter_context(tc.tile_pool(name="nrm", bufs=2))
        ysb = ctx.enter_context(tc.tile_pool(name="ysb", bufs=4))
        prt = ctx.enter_context(tc.tile_pool(name="prt", bufs=8))
        dbp = ctx.enter_context(tc.tile_pool(name="dbp", bufs=2, space="DRAM"))

        # HAM warm-up: ~18 matmuls on a zeroed tile keep the PE busy while
        # the first real operands stream in, so the clock gate is already
        # 8/8 when projection work starts.
        wz = singles.tile([P, QC], F16)
        nc.vector.memset(wz[:], 0.0)
        ones1 = singles.tile([1, DK], F16)
        nc.vector.memset(ones1[:], 1.0)
        wp = pjps.tile([P, QC], F32, tag="pj", name="warmps")
        for r in range(15):
            nc.tensor.matmul(wp[:], lhsT=wz[:, 0:P], rhs=wz[:],
                             start=True, stop=True)

        yTr = yT.rearrange("(a p) t -> p a t", p=P)

        # --- filler generators (one call = one PE psum-group + its evac) ---
        def proj_group_q(tci, et, dst, b_sb, w_sb, nm):
            tsl = slice(tci * QC, (tci + 1) * QC)
            ps = pjps.tile([P, QC], F32, tag="pj", name=f"ps_{nm}{tci}{et}")
            xc = x_ch[(nm, tci)]
            for ft in range(8):
                nc.tensor.matmul(ps[:],
                                 lhsT=w_sb[:, ft, et * P:(et + 1) * P],
                                 rhs=xc[:, ft, :],
                                 start=(ft == 0), stop=(ft == 7))
            nc.vector.tensor_scalar_add(out=dst[:, et, tsl], in0=ps[:],
                                        scalar1=b_sb[:, et:et + 1])

        def proj_group_v(tci, pair):
            # two key-tiles (pair=0 -> tt 0,1; pair=1 -> tt 2,3) in one
            # psum bank, evacuated by a single strided cast
            ps = pjps.tile([P, 2, EL], F32, tag="pj", name=f"ps_v{tci}{pair}")
            xc = x_ch[("v", tci)]
            for tt in range(2):
                for ft in range(8):
                    nc.tensor.matmul(
                        ps[:, tt, :],
                        lhsT=xc[:, ft, (2 * pair + tt) * P:(2 * pair + tt + 1) * P],
                        rhs=wv_sb[:, ft, :],
                        start=(ft == 0), stop=(ft == 7))
            ktg = tci * 4 + 2 * pair
            src = ps[:].rearrange("p a (h e) -> p a h e", h=HL)
            nc.vector.tensor_copy(out=Vaug[:, ktg:ktg + 2, :, 0:DK], in_=src)

        def f_q(tci, et):
            return lambda: proj_group_q(tci, et, Qt, bq_sb, wq_sb, "q")

        def f_k(tci, et):
            return lambda: proj_group_q(tci, et, Kt, bk_sb, wk_sb, "k")

        def f_v(tci, pair):
            return lambda: proj_group_v(tci, pair)

        def yproj_group(qc, mt, xTt):
            yp = pjps.tile([P, QC], F32, tag="pj", name=f"yp{qc}{mt}")
            for ct in range(2):
                nc.tensor.matmul(yp[:],
                                 lhsT=wo_sb[:, ct, mt * P:(mt + 1) * P],
                                 rhs=xTt[:, ct, :],
                                 start=(ct == 0), stop=(ct == 1))
            ys = ysb.tile([P, QC], F16, tag="ys", name=f"ys{qc}{mt}")
            if mt < Y_ON_SCALAR:
                nc.scalar.activation(out=ys[:], in_=yp[:],
                                     func=mybir.ActivationFunctionType.Identity,
                                     bias=yb_sb[:, mt:mt + 1])
            else:
                nc.vector.tensor_scalar_add(out=ys[:], in0=yp[:],
                                            scalar1=yb_sb[:, mt:mt + 1])
            nc.sync.dma_start(out=yTr[:, mt, qc * QC:(qc + 1) * QC], in_=ys[:])

        def f_y(qc, mt):
            return lambda: yproj_group(qc, mt, xTts[qc])

        # --- attention for one (qc, hp) with PE filler interleave ---
        def attention_hp(qc, hp, xTt, fillers):
            tiles = plan[qc]
            et = hp
            ntile = len(tiles)
            av = avy.tile([P, 2, QC], F32, tag="avy", name=f"av{qc}{hp}")
            ptbs = []

            def emit_av(ti, t):
                for hh in range(2):
                    nc.tensor.matmul(
                        av[0:DK + 1, hh, t.s0:t.s1],
                        lhsT=Vaug[:, t.kt, 2 * hp + hh, :],
                        rhs=ptbs[ti][:, hh, t.s0:t.s1],
                        start=t.first, stop=t.last,
                        skip_group_check=True)

            emitted = 0
            for ti, t in enumerate(tiles):
                ps = spool.tile([P, 2, QC], F32, tag="s",
                                name=f"s{qc}{hp}{ti}")
                for hh in range(2):
                    po = hh * DK
                    nc.tensor.matmul(
                        ps[:, hh, t.s0:t.s1],
                        lhsT=Kt[po:po + DK, et, t.kt * KT:(t.kt + 1) * KT],
                        rhs=Qt[po:po + DK, et, qc * QC + t.s0:qc * QC + t.s1],
                        start=True, stop=True)
                if ti > 0:
                    emit_av(ti - 1, tiles[ti - 1])
                # one exp covering both heads' identical column ranges
                pt = ptp.tile([P, 2, QC], F16, tag="pt", name=f"pt{qc}{hp}{ti}")
                ptbs.append(pt)
                nc.scalar.activation(
                    out=pt[:, :, t.s0:t.s1], in_=ps[:, :, t.s0:t.s1],
                    func=mybir.ActivationFunctionType.Exp, scale=float(SCALE))
                # mixed blocks: one masked multiply for both heads
                for qt, pid in t.muls:
                    sl = slice(qt * KT, (qt + 1) * KT)
                    pm = pat_sb[:, pid, :].rearrange(
                        "p k -> p () k").broadcast_to((P, 2, KT))
                    nc.vector.tensor_tensor(
                        out=pt[:, :, sl], in0=pt[:, :, sl], in1=pm,
                        op=mybir.AluOpType.mult)
                # PE filler budget: spread fillers evenly across tiles
                want = len(fillers) * (ti + 1) // ntile
                while emitted < want:
                    fillers[emitted]()
                    emitted += 1
            emit_av(ntile - 1, tiles[-1])
            return av

        # --- normalize: evacuate av, reciprocal rows via ScalarE ln/exp ---
        def norm_evac(qc, hp, av):
            """copy av out of PSUM (frees the accumulator banks for the next
            head pair), turn the denominator rows into reciprocals with
            ln/exp on ScalarE (same activation-table set as the softmax
            exp), then broadcast them across partitions via one DRAM
            round-trip."""
            cpb = nrm.tile([DK + 1, 2, QC], F32, tag="cp",
                           name=f"cp{qc}{hp}")
            nc.vector.tensor_copy(out=cpb[:], in_=av[0:DK + 1, :, :])
            rr = nrm.tile([1, 2, QC], F32, tag="rr", name=f"rr{qc}{hp}")
            nc.scalar.activation(out=rr[:], in_=av[DK:DK + 1, :, :],
                                 func=mybir.ActivationFunctionType.Ln)
            nc.scalar.activation(out=rr[:], in_=rr[:],
                                 func=mybir.ActivationFunctionType.Exp,
                                 scale=-1.0)
            dnb = dbp.tile([P, 8], F32, tag="dnb", name=f"dnb{qc}{hp}")
            nc.gpsimd.dma_start(
                out=dnb[:].rearrange("p j -> () (p j)"),
                in_=rr[:].rearrange("o h q -> o (h q)"))
            bc = nrm.tile([DK, 2, QC], F32, tag="bc", name=f"bc{qc}{hp}")
            dnf = dnb[:].rearrange("p j -> () (p j)").rearrange(
                "o (h q) -> o h q", h=2)
            nc.gpsimd.dma_start(out=bc[:], in_=dnf[0:1].partition_broadcast(DK))
            return [cpb[0:DK, 0, :], cpb[0:DK, 1, :]], bc

        def norm_mult(qc, hp, xTt, in0s, bc):
            et = hp
            nc.vector.tensor_tensor(
                out=xTt[0:DK, et, :], in0=in0s[0], in1=bc[:, 0, :],
                op=mybir.AluOpType.mult)
            tmp = nrm.tile([DK, QC], F16, tag="tmp", name=f"tm{qc}{hp}")
            nc.vector.tensor_tensor(
                out=tmp[:], in0=in0s[1], in1=bc[:, 1, :],
                op=mybir.AluOpType.mult)
            nc.scalar.dma_start(out=xTt[DK:P, et, :], in_=tmp[:])

        # --- global filler schedule -----------------------------------
        # Q proj of chunk c must land before qc c starts; K proj of chunk c
        # before its scores reach key-tile 4c; V proj of chunk c before its
        # A@V reaches key-tile 4c.  That lets K/V work slide late, feeding
        # the exp-bound final chunks where attention alone can't keep the
        # PE busy.  yproj(c) slides anywhere after chunk c's normalize.
        xTts = {}
        sched = {
            (0, 0): [f_v(0, 0), f_v(0, 1), f_q(0, 1), f_k(0, 1),
                     f_q(1, 0), f_q(1, 1)],
            (0, 1): [f_k(1, 0), f_k(1, 1)],
            (1, 0): [f_v(1, 0), f_v(1, 1), f_q(2, 0), f_q(2, 1)],
            (1, 1): [f_k(2, 0), f_k(2, 1), f_y(0, 0), f_y(0, 1),
                     f_y(0, 2), f_y(0, 3), f_y(0, 4), f_y(0, 5)],
            (2, 0): [f_v(2, 0), f_v(2, 1), f_q(3, 0), f_q(3, 1),
                     f_y(0, 6), f_y(0, 7)],
            (2, 1): [f_k(3, 0), f_k(3, 1), f_y(1, 0), f_y(1, 1),
                     f_y(1, 2), f_y(1, 3)],
            (3, 0): [f_v(3, 0), f_v(3, 1), f_y(1, 4), f_y(1, 5),
                     f_y(1, 6), f_y(1, 7)],
        }

        # qc3.hp1 additionally computes the ct=0 halves of yproj(3) into
        # SBUF partials (the tail then only needs the ct=1 matmuls), and
        # the (3,0) normalize multiply rides along as a filler once its
        # broadcast has landed
        nm30 = {}
        parts3 = {}

        def f_mult30():
            return lambda: norm_mult(3, 0, *nm30["args"])

        def f_ypart3(mt):
            def go():
                yp = pjps.tile([P, QC], F32, tag="pj", name=f"yp3a{mt}")
                nc.tensor.matmul(yp[:],
                                 lhsT=wo_sb[:, 0, mt * P:(mt + 1) * P],
                                 rhs=xTts[3][:, 0, :],
                                 start=True, stop=True)
                part = prt.tile([P, QC], F16, tag="part", name=f"part{mt}")
                nc.vector.tensor_copy(out=part[:], in_=yp[:])
                parts3[mt] = part
            return go

        sched[(3, 1)] = ([f_y(2, mt) for mt in range(6)] + [f_mult30()] +
                         [f_y(2, 6), f_y(2, 7)] +
                         [f_ypart3(mt) for mt in range(NMT)])

        # minimal prologue projections: just what qc0.hp0 needs up front
        proj_group_q(0, 0, Qt, bq_sb, wq_sb, "q")
        proj_group_q(0, 0, Kt, bk_sb, wk_sb, "k")

        # --- main pipelined loop ---
        # a normalize multiply waits on its 4-hop DMA chain, so it is
        # emitted one half-chunk after its norm_evac: (qc-1,hp1)'s multiply
        # goes between (qc,hp0) and (qc,hp1); (qc,hp0)'s goes after
        # (qc,hp1).  yproj fillers for a chunk only appear after its hp1
        # multiply has been emitted.
        prev_hp1 = None          # (qc, 1, xTt, in0s, bc) of the previous chunk
        for qc in range(NQC):
            xTt = xtp.tile([P, 2, QC], F16, tag="xT", name=f"xT{qc}")
            xTts[qc] = xTt

            av0 = attention_hp(qc, 0, xTt, sched[(qc, 0)])
            in0s0, bc0 = norm_evac(qc, 0, av0)
            if prev_hp1 is not None:
                norm_mult(*prev_hp1)
                prev_hp1 = None
            if qc == NQC - 1:
                nm30["args"] = (xTt, in0s0, bc0)

            av1 = attention_hp(qc, 1, xTt, sched[(qc, 1)])
            if qc < NQC - 1:
                in0s1, bc1 = norm_evac(qc, 1, av1)
                norm_mult(qc, 0, xTt, in0s0, bc0)
                prev_hp1 = (qc, 1, xTt, in0s1, bc1)
            else:
                # tail normalize: reciprocal rows straight to fp16, then
                # broadcast with a K=1 ones-matmul into PSUM instead of the
                # ~6us DRAM round trip (the PE is idle here anyway)
                cpb = nrm.tile([DK + 1, 2, QC], F32, tag="cp", name="cpT")
                nc.vector.tensor_copy(out=cpb[:], in_=av1[0:DK + 1, :, :])
                rr = nrm.tile([1, 2, QC], F32, tag="rr", name="rrT")
                nc.scalar.activation(out=rr[:], in_=av1[DK:DK + 1, :, :],
                                     func=mybir.ActivationFunctionType.Ln)
                rr16 = nrm.tile([1, 2, QC], F16, tag="rr16", name="rrT16")
                nc.scalar.activation(out=rr16[:], in_=rr[:],
                                     func=mybir.ActivationFunctionType.Exp,
                                     scale=-1.0)
                # keep the PE clock warm while ln/exp run
                wpt = pjps.tile([P, QC], F32, tag="pj", name="tailwarm")
                for r in range(8):
                    nc.tensor.matmul(wpt[:], lhsT=wz[:, 0:P], rhs=wz[:],
                                     start=True, stop=True)
                bcp = avy.tile([P, 2, QC], F32, tag="avy", name="bcT")
                for hh in range(2):
                    nc.tensor.matmul(bcp[0:DK, hh, :], lhsT=ones1[:],
                                     rhs=rr16[:, hh, :], start=True, stop=True)
                for r in range(8):
                    nc.tensor.matmul(wpt[:], lhsT=wz[:, 0:P], rhs=wz[:],
                                     start=True, stop=True)
                tail_tmp = nrm.tile([DK, QC], F16, tag="tmp", name="tmT")
                nc.vector.tensor_tensor(
                    out=tail_tmp[:], in0=cpb[0:DK, 1, :], in1=bcp[0:DK, 1, :],
                    op=mybir.AluOpType.mult)
                nc.vector.tensor_tensor(
                    out=xTt[0:DK, 1, :], in0=cpb[0:DK, 0, :],
                    in1=bcp[0:DK, 0, :], op=mybir.AluOpType.mult)

        # tail: ct=1 matmuls of the last chunk's y projection, folded with
        # the precomputed ct=0 partials
        xT3 = xTts[NQC - 1]
        ysp = None
        for mt in range(NMT):
            if mt % 2 == 0:
                yp = pjps.tile([P, QC], F32, tag="pj", name=f"yp3b{mt}")
                ysp = ysb.tile([P, 2, QC], F16, tag="ys2", name=f"ys3p{mt}")
            else:
                ypw = spool.tile([P, 2, QC], F32, tag="s", name=f"yp3s{mt}")
                yp = ypw[:, 0, :]
            nc.tensor.matmul(yp[:],
                             lhsT=wo_sb[0:DK, 1, mt * P:(mt + 1) * P],
                             rhs=xT3[0:DK, 1, :],
                             start=True, stop=False)
            nc.tensor.matmul(yp[:],
                             lhsT=woB[:, mt * P:(mt + 1) * P],
                             rhs=tail_tmp[:],
                             start=False, stop=True)
            nc.vector.scalar_tensor_tensor(
                out=ysp[:, mt % 2, :], in0=yp[:], scalar=yb_sb[:, mt:mt + 1],
                in1=parts3[mt][:], op0=mybir.AluOpType.add,
                op1=mybir.AluOpType.add)
            if mt % 2 == 1:
                nc.sync.dma_start(
                    out=yTr[:, mt - 1:mt + 1, (NQC - 1) * QC:NQC * QC],
                    in_=ysp[:])

    _legalize_waits(nc)
    return nc


# ----- SPMD runner ----------------------------------------------------------
_NEFF_MEMO = {}


def _install_memo_hook():
    import libneuronxla
    from concourse.bass2jax import install_neuronx_cc_hook

    install_neuronx_cc_hook()
    inner = libneuronxla.neuronx_cc
    if getattr(inner, "_is_memo_hook", False):
        return

    def memo_hook(code, code_format, platform_version, file_prefix):
        import hashlib
        key = hashlib.sha256(bytes(code)).hexdigest()
        if key not in _NEFF_MEMO:
            _NEFF_MEMO[key] = inner(code, code_format, platform_version,
                                    file_prefix)
        return _NEFF_MEMO[key]

    memo_hook._is_memo_hook = True
    libneuronxla.neuronx_cc = memo_hook


def run_spmd(nc, in_maps):
    import jax
    from concourse.bass2jax import _bass_exec_p

    _install_memo_hook()
    n_cores = len(in_maps)
    partition_name = (nc.partition_id_tensor.name
                      if nc.partition_id_tensor is not None else None)
    in_names, out_names, out_avals = [], [], []
    for alloc in nc.m.functions[0].allocations:
        if not isinstance(alloc, mybir.MemoryLocationSet):
            continue
        name = alloc.memorylocations[0].name
        if alloc.kind == "ExternalInput":
            if name != partition_name:
                in_names.append(name)
        elif alloc.kind == "ExternalOutput":
            out_names.append(name)
            out_avals.append(jax.core.ShapedArray(
                tuple(alloc.tensor_shape), mybir.dt.np(alloc.dtype)))
    bind_in_names = tuple(in_names +
                          ([partition_name] if partition_name else []))

    def _body(*args):
        return tuple(_bass_exec_p.bind(
            *args, out_avals=tuple(out_avals), in_names=bind_in_names,
            out_names=tuple(out_names), lowering_input_output_aliases=(),
            sim_require_finite=True, sim_require_nnan=True, nc=nc))

    devices = jax.devices()[:n_cores]
    f = jax.jit(_body)
    futs = []
    for c in range(n_cores):
        args = [jax.device_put(np.asarray(in_maps[c][nm]), devices[c])
                for nm in in_names]
        if partition_name:
            args.append(jax.device_put(np.array([[c]], np.uint32), devices[c]))
        futs.append(f(*args))
    return [{nm: np.asarray(futs[c][i]) for i, nm in enumerate(out_names)}
            for c in range(n_cores)]


# ----- host wrapper ---------------------------------------------------------
_CACHE = {}


def _get_program(mask):
    key = mask.tobytes()
    if key not in _CACHE:
        plan, pats = _mask_plan(mask)
        nc = build_program(plan, pats.shape[0])
        _CACHE[key] = (nc, pats)
    return _CACHE[key]


def _sw_x(xT):
    """[D, S] -> [P, NQC, 8, QC] so each (partition, chunk) is contiguous"""
    return np.ascontiguousarray(
        xT.reshape(8, P, NQC, QC).transpose(1, 2, 0, 3))


def _sw_w(wT, a):
    """[a*P, cols] -> [P, a, cols]"""
    return np.ascontiguousarray(
        wT.reshape(a, P, wT.shape[1]).transpose(1, 0, 2))


def make_in_maps(q, k, v, mask, wq, bq, wk, bk, wv, bv, wo, bo, pats):
    q, k, v = (np.asarray(a, np.float32) for a in (q, k, v))
    in_maps = []
    for c in range(NCORES):
        b, g = divmod(c, TP)
        sl = slice(g * EL, (g + 1) * EL)
        woT_g = np.ascontiguousarray(wo[:, sl].T)        # [EL, D]
        in_maps.append({
            "xqT": _sw_x(q[b].T.astype(F16NP)),
            "xkT": _sw_x(k[b].T.astype(F16NP)),
            "xvT": _sw_x(v[b].T.astype(F16NP)),
            "wqT": _sw_w(wq[sl, :].T.astype(F16NP), 8),
            "wkT": _sw_w(wk[sl, :].T.astype(F16NP), 8),
            "wvT": _sw_w(wv[sl, :].T.astype(F16NP), 8),
            "woT": _sw_w(woT_g.astype(F16NP), 2),
            "bq2": np.ascontiguousarray(bq[sl].reshape(2, P).T),
            "bk2": np.ascontiguousarray(bk[sl].reshape(2, P).T),
            "ybias": np.ascontiguousarray(
                (bv[sl].astype(np.float64) @ woT_g.astype(np.float64))
                .astype(np.float32).reshape(NMT, P).T),
            "pats": np.ascontiguousarray(pats.transpose(1, 0, 2)),
        })
    return in_maps


def assemble_output(results, bo):
    y = np.empty((B, S, D), np.float32)
    for b in range(B):
        acc = results[b * TP]["yT"].astype(np.float32)
        for g in range(1, TP):
            acc = acc + results[b * TP + g]["yT"].astype(np.float32)
        y[b] = acc.T + np.asarray(bo, np.float32)[None, :]
    return y


def kernel(q, k, v, mask, wq, bq, wk, bk, wv, bv, wo, bo):
    mask2d = np.asarray(mask).reshape(S, S)
    nc, pats = _get_program(mask2d)
    in_maps = make_in_maps(q, k, v, mask2d, wq, bq, wk, bk, wv, bv, wo, bo, pats)
    return assemble_output(run_spmd(nc, in_maps), bo)
